# revision 5
# baseline (speedup 1.0000x reference)
"""Trainium2 Bass kernel for nn_CSA_ConvBlock (conv-self-attention block).

Reference math (B,C,H,W = 16,256,64,64):
  fq = conv3x3(x, wq); fk = conv3x3(x, wk); fv = conv3x3(x, wv)
  q_sum = fq.sum(H); k_sum = fk.sum(C,H)
  f_scores[b,c] = sum_w q_sum[b,c,w]*k_sum[b,w] / (sqrt(W)*H^2)
  scores = softmax_C(f_scores)
  out = relu(BN_eval(scores*fv + x))

Key algebraic reduction: fq and fk are only consumed through H-sums, and
conv is linear, so q_sum/k_sum collapse to 3-tap-x-3-dy matmuls over the
column sums of x (with top/bottom row edge corrections for SAME padding).
Only conv(x, wv) is computed in full.  Since scores ~ 1/C ~ 0.004, the
attention branch is strongly suppressed relative to the fp32 residual x,
so bf16 matmuls are numerically safe.

Sharding: data-parallel over batch, 2 batches per core on 8 cores.
"""

import os
import sys
import numpy as np
from contextlib import ExitStack

if "/opt/trn_rl_repo" not in sys.path and not any(
    "trn_rl_repo" in p for p in sys.path
):
    sys.path.insert(0, "/opt/trn_rl_repo")

import concourse.bass as bass
import concourse.tile as tile
from concourse import bacc, mybir
from concourse import bass_utils
from concourse.masks import make_identity

B, C, H, W = 16, 256, 64, 64
NCORES = 8
BPC = B // NCORES          # batches per core
P = 128                    # partitions
KT = C // P                # channel k-tiles (2)
MT = C // P                # channel m-tiles (2)
PW = W + 2                 # padded width 66
PH = H + 2                 # padded height 66
NTAP = 9
ROWS_PER_ST = 8
NF = ROWS_PER_ST * W       # 512 free elems per spatial tile
ST = (H * W) // NF         # 8 spatial tiles per (batch, mtile)
EPS = 1e-5
SCORE_SCALE = 1.0 / (np.sqrt(np.float32(W)) * (H * H))  # 1/32768

FP32 = mybir.dt.float32
BF16 = mybir.dt.bfloat16
AX = mybir.AxisListType
ALU = mybir.AluOpType
ACTF = mybir.ActivationFunctionType


def _emit(ctx: ExitStack, tc: "tile.TileContext", nc, x, wq, wk, wv,
          gamma, beta, rmean, rvar, out):
    consts = ctx.enter_context(tc.tile_pool(name="consts", bufs=1))

    ident = consts.tile([P, P], FP32, tag="ident")
    make_identity(nc, ident[:])
    ones_col = consts.tile([P, 1], FP32, tag="ones")
    nc.gpsimd.memset(ones_col[:], 1.0)

    # ---- BN params to per-partition layout, fold into affine ----
    # inv = gamma / sqrt(var + eps);  bias2 = beta - mean*inv
    par = {}
    for name, ap in [("gamma", gamma), ("beta", beta),
                     ("mean", rmean), ("var", rvar)]:
        par[name] = []
        for mt in range(MT):
            t = consts.tile([P, 1], FP32, tag=f"{name}{mt}")
            nc.sync.dma_start(t[:], ap[mt * P:(mt + 1) * P])
            par[name].append(t)
    eps_t = consts.tile([P, 1], FP32, tag="eps")
    nc.gpsimd.memset(eps_t[:], EPS)
    inv_t, bias2_t = [], []
    for mt in range(MT):
        sq = consts.tile([P, 1], FP32, tag=f"sq{mt}")
        nc.scalar.activation(sq[:], par["var"][mt][:], ACTF.Sqrt, bias=eps_t[:])
        rec = consts.tile([P, 1], FP32, tag=f"rec{mt}")
        nc.vector.reciprocal(rec[:], sq[:])
        iv = consts.tile([P, 1], FP32, tag=f"inv{mt}")
        nc.vector.tensor_mul(iv[:], rec[:], par["gamma"][mt][:])
        inv_t.append(iv)
        mi = consts.tile([P, 1], FP32, tag=f"mi{mt}")
        nc.vector.tensor_mul(mi[:], par["mean"][mt][:], iv[:])
        b2 = consts.tile([P, 1], FP32, tag=f"b2{mt}")
        nc.vector.tensor_sub(b2[:], par["beta"][mt][:], mi[:])
        bias2_t.append(b2)

    # ---- weight prep ----
    # wqT/wvT: per k-tile [i=128, (tap, o=256)] bf16 (PE-transposed per tap)
    # wks:     per k-tile [i=128, tap] bf16 -- wk summed over output channels
    wT_pool = ctx.enter_context(tc.tile_pool(name="wT", bufs=1))
    wqT = [wT_pool.tile([P, NTAP * C], BF16, tag=f"wqT{kt}", name=f"wqT{kt}")
           for kt in range(KT)]
    wvT = [wT_pool.tile([P, NTAP * C], BF16, tag=f"wvT{kt}", name=f"wvT{kt}")
           for kt in range(KT)]
    wks = [wT_pool.tile([P, NTAP], BF16, tag=f"wks{kt}", name=f"wks{kt}")
           for kt in range(KT)]

    with tc.tile_pool(name="wraw", bufs=3) as wraw_pool, \
         tc.tile_pool(name="prep_psum", bufs=4, space="PSUM") as prep_psum:
        for wap, dest in [(wq, wqT), (wv, wvT)]:
            for ot in range(MT):
                raw = wraw_pool.tile([P, C * NTAP], FP32, tag="wraw")
                nc.sync.dma_start(raw[:], wap[ot * P:(ot + 1) * P])
                r3 = raw[:].rearrange("o (i t) -> o i t", t=NTAP)
                for kt in range(KT):
                    for tap in range(NTAP):
                        pt = prep_psum.tile([P, P], FP32, tag="tp")
                        nc.tensor.transpose(
                            pt[:], r3[:, kt * P:(kt + 1) * P, tap], ident[:])
                        nc.vector.tensor_copy(
                            dest[kt][:, tap * C + ot * P: tap * C + (ot + 1) * P],
                            pt[:])
        # wk: column-sum over all 256 output channels per (i, tap)
        rawk = []
        for ot in range(MT):
            rk = wraw_pool.tile([P, C * NTAP], FP32, tag="wraw")
            nc.sync.dma_start(rk[:], wk[ot * P:(ot + 1) * P])
            rawk.append(rk)
        for kt in range(KT):
            for tap in range(NTAP):
                pk = prep_psum.tile([P, 1], FP32, tag="pk")
                for ot in range(MT):
                    r3 = rawk[ot][:].rearrange("o (i t) -> o i t", t=NTAP)
                    nc.tensor.matmul(
                        pk[:], r3[:, kt * P:(kt + 1) * P, tap], ones_col[:],
                        start=(ot == 0), stop=(ot == MT - 1))
                nc.vector.tensor_copy(wks[kt][:, tap:tap + 1], pk[:])

    # ---- main per-batch pipeline ----
    xf32_pool = ctx.enter_context(tc.tile_pool(name="xf32", bufs=2 * KT))
    xpad_pool = ctx.enter_context(tc.tile_pool(name="xpad", bufs=2 * KT))
    agg_pool = ctx.enter_context(tc.tile_pool(name="agg", bufs=2 * KT))
    small = ctx.enter_context(tc.tile_pool(name="small", bufs=2))
    ev_pool = ctx.enter_context(tc.tile_pool(name="ev", bufs=3))
    qk_psum = ctx.enter_context(tc.tile_pool(name="qk_psum", bufs=2, space="PSUM"))
    misc_psum = ctx.enter_context(tc.tile_pool(name="misc_psum", bufs=1, space="PSUM"))
    fv_psum = ctx.enter_context(tc.tile_pool(name="fv_psum", bufs=4, space="PSUM"))

    for b in range(BPC):
        # load x (fp32, flat) per k-tile
        xf = []
        for kt in range(KT):
            t = xf32_pool.tile([P, H * W], FP32, tag="xf32")
            nc.sync.dma_start(t[:], x[b, kt * P:(kt + 1) * P])
            xf.append(t)

        # padded bf16 image + column-sum aggregates
        xp, aggs = [], []
        for kt in range(KT):
            tp = xpad_pool.tile([P, PH * PW], BF16, tag="xpad")
            nc.gpsimd.memset(tp[:], 0.0)
            nc.vector.tensor_copy(
                tp[:].rearrange("p (r c) -> p r c", c=PW)[:, 1:H + 1, 1:W + 1],
                xf[kt][:].rearrange("p (h w) -> p h w", w=W))
            xp.append(tp)

            cs = small.tile([P, W], FP32, tag="cs")
            nc.vector.tensor_reduce(
                cs[:], xf[kt][:].rearrange("p (h w) -> p w h", w=W),
                axis=AX.X, op=ALU.add)
            xhw = xf[kt][:].rearrange("p (h w) -> p h w", w=W)
            ag = agg_pool.tile([P, 3 * PW], BF16, tag="agg")
            nc.gpsimd.memset(ag[:], 0.0)
            a3 = ag[:].rearrange("p (a c) -> p a c", c=PW)
            # dy=0 row-window is rows -1..H-2: colsum - bottom row
            nc.vector.tensor_sub(a3[:, 0, 1:W + 1], cs[:], xhw[:, H - 1, :])
            nc.vector.tensor_copy(a3[:, 1, 1:W + 1], cs[:])
            # dy=2 row-window is rows 1..H: colsum - top row
            nc.vector.tensor_sub(a3[:, 2, 1:W + 1], cs[:], xhw[:, 0, :])
            aggs.append(ag)

        # q_sum (per mtile) and k_sum into one PSUM tile:
        # cols [mt*W:(mt+1)*W] = q_sum[mt]; row 0 cols [2W:3W] = k_sum
        qk = qk_psum.tile([P, 3 * W], FP32, tag="qk")
        for mt in range(MT):
            idx = 0
            for kt in range(KT):
                a3 = aggs[kt][:].rearrange("p (a c) -> p a c", c=PW)
                for tap in range(NTAP):
                    dy, dx = divmod(tap, 3)
                    nc.tensor.matmul(
                        qk[:, mt * W:(mt + 1) * W],
                        wqT[kt][:, tap * C + mt * P: tap * C + mt * P + P],
                        a3[:, dy, dx:dx + W],
                        start=(idx == 0), stop=(idx == KT * NTAP - 1))
                    idx += 1
        idx = 0
        for kt in range(KT):
            a3 = aggs[kt][:].rearrange("p (a c) -> p a c", c=PW)
            for tap in range(NTAP):
                dy, dx = divmod(tap, 3)
                nc.tensor.matmul(
                    qk[0:1, 2 * W:3 * W],
                    wks[kt][:, tap:tap + 1],
                    a3[:, dy, dx:dx + W],
                    start=(idx == 0), stop=(idx == KT * NTAP - 1))
                idx += 1

        # f_scores[c] = dot(q_sum[c,:], k_sum) (scale folded into softmax)
        ksb = small.tile([1, W], FP32, tag="ksb")
        nc.vector.tensor_copy(ksb[:], qk[0:1, 2 * W:3 * W])
        kb = small.tile([P, W], FP32, tag="kb")
        nc.gpsimd.partition_broadcast(kb[:], ksb[:])
        fs = []
        for mt in range(MT):
            scr = small.tile([P, W], FP32, tag=f"scr{mt}")
            nc.vector.tensor_mul(scr[:], qk[:, mt * W:(mt + 1) * W], kb[:])
            f = small.tile([P, 1], FP32, tag=f"fs{mt}")
            nc.vector.tensor_reduce(f[:], scr[:], axis=AX.X, op=ALU.add)
            fs.append(f)

        # softmax over C=256 in a [1, 256] row
        fsrow = misc_psum.tile([1, C], FP32, tag="fsrow")
        for mt in range(MT):
            nc.tensor.transpose(fsrow[0:1, mt * P:(mt + 1) * P], fs[mt][:], ident[:])
        mx = small.tile([1, 1], FP32, tag="mx")
        nc.vector.tensor_reduce(mx[:], fsrow[:], axis=AX.X, op=ALU.max)
        mxs = small.tile([1, 1], FP32, tag="mxs")
        nc.vector.tensor_scalar_mul(mxs[:], mx[:], -float(SCORE_SCALE))
        es = small.tile([1, C], FP32, tag="es")
        nc.scalar.activation(es[:], fsrow[:], ACTF.Exp,
                             bias=mxs[:], scale=float(SCORE_SCALE))
        ssum = small.tile([1, 1], FP32, tag="ssum")
        nc.vector.tensor_reduce(ssum[:], es[:], axis=AX.X, op=ALU.add)
        rs = small.tile([1, 1], FP32, tag="rs")
        nc.vector.reciprocal(rs[:], ssum[:])
        srow = small.tile([1, C], FP32, tag="srow")
        nc.vector.tensor_scalar_mul(srow[:], es[:], rs[:])

        # scores back to [128,1] per mtile (K=1 matmul), fold in BN inv
        s1 = []
        for mt in range(MT):
            stp = misc_psum.tile([P, 1], FP32, tag="stp")
            nc.tensor.matmul(stp[:], srow[:, mt * P:(mt + 1) * P],
                             ones_col[0:1, 0:1], start=True, stop=True)
            t = small.tile([P, 1], FP32, tag=f"s1{mt}")
            nc.vector.tensor_mul(t[:], stp[:], inv_t[mt][:])
            s1.append(t)

        # fv conv (18 accumulating matmuls per [128,512] tile) + fused
        # eviction: out = relu(fv*s1 + (x*inv + bias2))
        for mt in range(MT):
            for st in range(ST):
                y0 = st * ROWS_PER_ST
                pv = fv_psum.tile([P, NF], FP32, tag="fv")
                idx = 0
                for kt in range(KT):
                    x3 = xp[kt][:].rearrange("p (r c) -> p r c", c=PW)
                    for tap in range(NTAP):
                        dy, dx = divmod(tap, 3)
                        nc.tensor.matmul(
                            pv[:],
                            wvT[kt][:, tap * C + mt * P: tap * C + mt * P + P],
                            x3[:, y0 + dy:y0 + dy + ROWS_PER_ST, dx:dx + W],
                            start=(idx == 0), stop=(idx == KT * NTAP - 1))
                        idx += 1
                at = ev_pool.tile([P, NF], FP32, tag="A")
                nc.scalar.activation(
                    at[:], xf[mt][:, st * NF:(st + 1) * NF], ACTF.Identity,
                    bias=bias2_t[mt][:], scale=inv_t[mt][:])
                rt = ev_pool.tile([P, NF], FP32, tag="r")
                nc.vector.scalar_tensor_tensor(
                    rt[:], pv[:], s1[mt][:], at[:],
                    op0=ALU.mult, op1=ALU.add)
                o_t = ev_pool.tile([P, NF], FP32, tag="o")
                nc.scalar.activation(o_t[:], rt[:], ACTF.Relu)
                nc.sync.dma_start(
                    out[b, mt * P:(mt + 1) * P].rearrange(
                        "c h w -> c (h w)")[:, st * NF:(st + 1) * NF],
                    o_t[:])


def build_nc():
    nc = bacc.Bacc("TRN2", target_bir_lowering=False, debug=False,
                   num_devices=NCORES)
    x = nc.dram_tensor("x", [BPC, C, H, W], FP32, kind="ExternalInput").ap()
    wq = nc.dram_tensor("wq", [C, C, 3, 3], FP32, kind="ExternalInput").ap()
    wk = nc.dram_tensor("wk", [C, C, 3, 3], FP32, kind="ExternalInput").ap()
    wv = nc.dram_tensor("wv", [C, C, 3, 3], FP32, kind="ExternalInput").ap()
    gamma = nc.dram_tensor("gamma", [C], FP32, kind="ExternalInput").ap()
    beta = nc.dram_tensor("beta", [C], FP32, kind="ExternalInput").ap()
    rmean = nc.dram_tensor("running_mean", [C], FP32, kind="ExternalInput").ap()
    rvar = nc.dram_tensor("running_var", [C], FP32, kind="ExternalInput").ap()
    out = nc.dram_tensor("out", [BPC, C, H, W], FP32, kind="ExternalOutput").ap()
    with tile.TileContext(nc) as tc, ExitStack() as ctx:
        _emit(ctx, tc, nc, x, wq, wk, wv, gamma, beta, rmean, rvar, out)
    nc.compile()
    return nc


_NC_CACHE = None


def _get_nc():
    global _NC_CACHE
    if _NC_CACHE is None:
        _NC_CACHE = build_nc()
    return _NC_CACHE


def make_in_maps(inputs: dict) -> list:
    rep = {k: np.ascontiguousarray(np.asarray(inputs[k], dtype=np.float32))
           for k in ("wq", "wk", "wv", "gamma", "beta",
                     "running_mean", "running_var")}
    xfull = np.ascontiguousarray(np.asarray(inputs["x"], dtype=np.float32))
    in_maps = []
    for c in range(NCORES):
        m = dict(rep)
        m["x"] = xfull[c * BPC:(c + 1) * BPC]
        in_maps.append(m)
    return in_maps


def kernel(**inputs) -> np.ndarray:
    nc = _get_nc()
    in_maps = make_in_maps(inputs)
    res = bass_utils.run_bass_kernel_spmd(nc, in_maps,
                                          core_ids=list(range(NCORES)))
    return np.concatenate([res.results[c]["out"] for c in range(NCORES)],
                          axis=0).astype(np.float32)


# revision 7
# speedup vs baseline: 63.0561x; 63.0561x over previous
"""Trainium2 Bass kernel for nn_CSA_ConvBlock (conv-self-attention block).

Reference math (B,C,H,W = 16,256,64,64):
  fq = conv3x3(x, wq); fk = conv3x3(x, wk); fv = conv3x3(x, wv)
  q_sum = fq.sum(H); k_sum = fk.sum(C,H)
  f_scores[b,c] = sum_w q_sum[b,c,w]*k_sum[b,w] / (sqrt(W)*H^2)
  scores = softmax_C(f_scores)
  out = relu(BN_eval(scores*fv + x))

Key algebraic reduction: fq and fk are only consumed through H-sums, and
conv is linear, so q_sum/k_sum collapse to 3-tap-x-3-dy matmuls over the
column sums of x (with top/bottom row edge corrections for SAME padding).
Only conv(x, wv) is computed in full.  Since scores ~ 1/C ~ 0.004, the
attention branch is strongly suppressed relative to the fp32 residual x,
so bf16 matmuls are numerically safe.

Sharding: data-parallel over batch, 2 batches per core on 8 cores.
"""

import os
import sys
import numpy as np
from contextlib import ExitStack

if "/opt/trn_rl_repo" not in sys.path and not any(
    "trn_rl_repo" in p for p in sys.path
):
    sys.path.insert(0, "/opt/trn_rl_repo")

import concourse.bass as bass
import concourse.tile as tile
from concourse import bacc, mybir
from concourse import bass_utils
from concourse.masks import make_identity

B, C, H, W = 16, 256, 64, 64
NCORES = 8
BPC = B // NCORES          # batches per core
P = 128                    # partitions
KT = C // P                # channel k-tiles (2)
MT = C // P                # channel m-tiles (2)
PW = W + 2                 # padded width 66
PH = H + 2                 # padded height 66
NTAP = 9
ROWS_PER_ST = 8
NF = ROWS_PER_ST * W       # 512 free elems per spatial tile
ST = (H * W) // NF         # 8 spatial tiles per (batch, mtile)
EPS = 1e-5
SCORE_SCALE = 1.0 / (np.sqrt(np.float32(W)) * (H * H))  # 1/32768

FP32 = mybir.dt.float32
BF16 = mybir.dt.bfloat16
AX = mybir.AxisListType
ALU = mybir.AluOpType
ACTF = mybir.ActivationFunctionType


def _emit(ctx: ExitStack, tc: "tile.TileContext", nc, x, wq, wk, wv,
          gamma, beta, rmean, rvar, out):
    consts = ctx.enter_context(tc.tile_pool(name="consts", bufs=1))

    ident = consts.tile([P, P], FP32, tag="ident")
    make_identity(nc, ident[:])
    ones_col = consts.tile([P, 1], FP32, tag="ones")
    nc.gpsimd.memset(ones_col[:], 1.0)

    # ---- BN params to per-partition layout, fold into affine ----
    # inv = gamma / sqrt(var + eps);  bias2 = beta - mean*inv
    par = {}
    for name, ap in [("gamma", gamma), ("beta", beta),
                     ("mean", rmean), ("var", rvar)]:
        par[name] = []
        for mt in range(MT):
            t = consts.tile([P, 1], FP32, tag=f"{name}{mt}")
            nc.sync.dma_start(t[:], ap[mt * P:(mt + 1) * P])
            par[name].append(t)
    eps_t = consts.tile([P, 1], FP32, tag="eps")
    nc.gpsimd.memset(eps_t[:], EPS)
    inv_t, bias2_t = [], []
    for mt in range(MT):
        sq = consts.tile([P, 1], FP32, tag=f"sq{mt}")
        nc.scalar.activation(sq[:], par["var"][mt][:], ACTF.Sqrt, bias=eps_t[:])
        rec = consts.tile([P, 1], FP32, tag=f"rec{mt}")
        nc.vector.reciprocal(rec[:], sq[:])
        iv = consts.tile([P, 1], FP32, tag=f"inv{mt}")
        nc.vector.tensor_mul(iv[:], rec[:], par["gamma"][mt][:])
        inv_t.append(iv)
        mi = consts.tile([P, 1], FP32, tag=f"mi{mt}")
        nc.vector.tensor_mul(mi[:], par["mean"][mt][:], iv[:])
        b2 = consts.tile([P, 1], FP32, tag=f"b2{mt}")
        nc.vector.tensor_sub(b2[:], par["beta"][mt][:], mi[:])
        bias2_t.append(b2)

    # ---- weight prep ----
    # wqT/wvT: per k-tile [i=128, (tap, o=256)] bf16 (PE-transposed per tap)
    # wks:     per k-tile [i=128, tap] bf16 -- wk summed over output channels
    wT_pool = ctx.enter_context(tc.tile_pool(name="wT", bufs=1))
    wqT = [wT_pool.tile([P, NTAP * C], BF16, tag=f"wqT{kt}", name=f"wqT{kt}")
           for kt in range(KT)]
    wvT = [wT_pool.tile([P, NTAP * C], BF16, tag=f"wvT{kt}", name=f"wvT{kt}")
           for kt in range(KT)]
    wks = [wT_pool.tile([P, NTAP], BF16, tag=f"wks{kt}", name=f"wks{kt}")
           for kt in range(KT)]

    with tc.tile_pool(name="wraw", bufs=3) as wraw_pool, \
         tc.tile_pool(name="prep_psum", bufs=4, space="PSUM") as prep_psum:
        for wap, dest in [(wq, wqT), (wv, wvT)]:
            for ot in range(MT):
                raw = wraw_pool.tile([P, C * NTAP], FP32, tag="wraw")
                nc.sync.dma_start(raw[:], wap[ot * P:(ot + 1) * P])
                r3 = raw[:].rearrange("o (i t) -> o i t", t=NTAP)
                for kt in range(KT):
                    for tap in range(NTAP):
                        pt = prep_psum.tile([P, P], FP32, tag="tp")
                        nc.tensor.transpose(
                            pt[:], r3[:, kt * P:(kt + 1) * P, tap], ident[:])
                        nc.vector.tensor_copy(
                            dest[kt][:, tap * C + ot * P: tap * C + (ot + 1) * P],
                            pt[:])
        # wk: column-sum over all 256 output channels per (i, tap)
        rawk = []
        for ot in range(MT):
            rk = wraw_pool.tile([P, C * NTAP], FP32, tag="wraw")
            nc.sync.dma_start(rk[:], wk[ot * P:(ot + 1) * P])
            rawk.append(rk)
        for kt in range(KT):
            for tap in range(NTAP):
                pk = prep_psum.tile([P, 1], FP32, tag="pk")
                for ot in range(MT):
                    r3 = rawk[ot][:].rearrange("o (i t) -> o i t", t=NTAP)
                    nc.tensor.matmul(
                        pk[:], r3[:, kt * P:(kt + 1) * P, tap], ones_col[:],
                        start=(ot == 0), stop=(ot == MT - 1))
                nc.vector.tensor_copy(wks[kt][:, tap:tap + 1], pk[:])

    # ---- main per-batch pipeline ----
    xf32_pool = ctx.enter_context(tc.tile_pool(name="xf32", bufs=2 * KT))
    xpad_pool = ctx.enter_context(tc.tile_pool(name="xpad", bufs=2 * KT))
    agg_pool = ctx.enter_context(tc.tile_pool(name="agg", bufs=2 * KT))
    small = ctx.enter_context(tc.tile_pool(name="small", bufs=2))
    ev_pool = ctx.enter_context(tc.tile_pool(name="ev", bufs=3))
    qk_psum = ctx.enter_context(tc.tile_pool(name="qk_psum", bufs=2, space="PSUM"))
    misc_psum = ctx.enter_context(tc.tile_pool(name="misc_psum", bufs=1, space="PSUM"))
    fv_psum = ctx.enter_context(tc.tile_pool(name="fv_psum", bufs=4, space="PSUM"))

    for b in range(BPC):
        # load x (fp32, flat) per k-tile
        xf = []
        for kt in range(KT):
            t = xf32_pool.tile([P, H * W], FP32, tag="xf32")
            nc.sync.dma_start(t[:], x[b, kt * P:(kt + 1) * P])
            xf.append(t)

        # padded bf16 image + column-sum aggregates
        xp, aggs = [], []
        for kt in range(KT):
            tp = xpad_pool.tile([P, PH * PW], BF16, tag="xpad")
            nc.gpsimd.memset(tp[:], 0.0)
            nc.vector.tensor_copy(
                tp[:].rearrange("p (r c) -> p r c", c=PW)[:, 1:H + 1, 1:W + 1],
                xf[kt][:].rearrange("p (h w) -> p h w", w=W))
            xp.append(tp)

            cs = small.tile([P, W], FP32, tag="cs")
            nc.vector.tensor_reduce(
                cs[:], xf[kt][:].rearrange("p (h w) -> p w h", w=W),
                axis=AX.X, op=ALU.add)
            xhw = xf[kt][:].rearrange("p (h w) -> p h w", w=W)
            ag = agg_pool.tile([P, 3 * PW], BF16, tag="agg")
            nc.gpsimd.memset(ag[:], 0.0)
            a3 = ag[:].rearrange("p (a c) -> p a c", c=PW)
            # dy=0 row-window is rows -1..H-2: colsum - bottom row
            nc.vector.tensor_sub(a3[:, 0, 1:W + 1], cs[:], xhw[:, H - 1, :])
            nc.vector.tensor_copy(a3[:, 1, 1:W + 1], cs[:])
            # dy=2 row-window is rows 1..H: colsum - top row
            nc.vector.tensor_sub(a3[:, 2, 1:W + 1], cs[:], xhw[:, 0, :])
            aggs.append(ag)

        # q_sum (per mtile) and k_sum into one PSUM tile:
        # cols [mt*W:(mt+1)*W] = q_sum[mt]; row 0 cols [2W:3W] = k_sum
        qk = qk_psum.tile([P, 3 * W], FP32, tag="qk")
        for mt in range(MT):
            idx = 0
            for kt in range(KT):
                a3 = aggs[kt][:].rearrange("p (a c) -> p a c", c=PW)
                for tap in range(NTAP):
                    dy, dx = divmod(tap, 3)
                    nc.tensor.matmul(
                        qk[:, mt * W:(mt + 1) * W],
                        wqT[kt][:, tap * C + mt * P: tap * C + mt * P + P],
                        a3[:, dy, dx:dx + W],
                        start=(idx == 0), stop=(idx == KT * NTAP - 1))
                    idx += 1
        idx = 0
        for kt in range(KT):
            a3 = aggs[kt][:].rearrange("p (a c) -> p a c", c=PW)
            for tap in range(NTAP):
                dy, dx = divmod(tap, 3)
                nc.tensor.matmul(
                    qk[0:1, 2 * W:3 * W],
                    wks[kt][:, tap:tap + 1],
                    a3[:, dy, dx:dx + W],
                    start=(idx == 0), stop=(idx == KT * NTAP - 1))
                idx += 1

        # f_scores[c] = dot(q_sum[c,:], k_sum) (scale folded into softmax)
        ksb = small.tile([1, W], FP32, tag="ksb")
        nc.vector.tensor_copy(ksb[:], qk[0:1, 2 * W:3 * W])
        kb = small.tile([P, W], FP32, tag="kb")
        nc.gpsimd.partition_broadcast(kb[:], ksb[:])
        fs = []
        for mt in range(MT):
            scr = small.tile([P, W], FP32, tag=f"scr{mt}")
            nc.vector.tensor_mul(scr[:], qk[:, mt * W:(mt + 1) * W], kb[:])
            f = small.tile([P, 1], FP32, tag=f"fs{mt}")
            nc.vector.tensor_reduce(f[:], scr[:], axis=AX.X, op=ALU.add)
            fs.append(f)

        # softmax over C=256 in a [1, 256] row
        fsrow = misc_psum.tile([1, C], FP32, tag="fsrow")
        for mt in range(MT):
            nc.tensor.transpose(fsrow[0:1, mt * P:(mt + 1) * P], fs[mt][:], ident[:])
        mx = small.tile([1, 1], FP32, tag="mx")
        nc.vector.tensor_reduce(mx[:], fsrow[:], axis=AX.X, op=ALU.max)
        mxs = small.tile([1, 1], FP32, tag="mxs")
        nc.vector.tensor_scalar_mul(mxs[:], mx[:], -float(SCORE_SCALE))
        es = small.tile([1, C], FP32, tag="es")
        nc.scalar.activation(es[:], fsrow[:], ACTF.Exp,
                             bias=mxs[:], scale=float(SCORE_SCALE))
        ssum = small.tile([1, 1], FP32, tag="ssum")
        nc.vector.tensor_reduce(ssum[:], es[:], axis=AX.X, op=ALU.add)
        rs = small.tile([1, 1], FP32, tag="rs")
        nc.vector.reciprocal(rs[:], ssum[:])
        srow = small.tile([1, C], FP32, tag="srow")
        nc.vector.tensor_scalar_mul(srow[:], es[:], rs[:])

        # scores back to [128,1] per mtile (K=1 matmul), fold in BN inv
        s1 = []
        for mt in range(MT):
            stp = misc_psum.tile([P, 1], FP32, tag="stp")
            nc.tensor.matmul(stp[:], srow[:, mt * P:(mt + 1) * P],
                             ones_col[0:1, 0:1], start=True, stop=True)
            t = small.tile([P, 1], FP32, tag=f"s1{mt}")
            nc.vector.tensor_mul(t[:], stp[:], inv_t[mt][:])
            s1.append(t)

        # fv conv (18 accumulating matmuls per [128,512] tile) + fused
        # eviction: out = relu(fv*s1 + (x*inv + bias2))
        for mt in range(MT):
            for st in range(ST):
                y0 = st * ROWS_PER_ST
                pv = fv_psum.tile([P, NF], FP32, tag="fv")
                idx = 0
                for kt in range(KT):
                    x3 = xp[kt][:].rearrange("p (r c) -> p r c", c=PW)
                    for tap in range(NTAP):
                        dy, dx = divmod(tap, 3)
                        nc.tensor.matmul(
                            pv[:],
                            wvT[kt][:, tap * C + mt * P: tap * C + mt * P + P],
                            x3[:, y0 + dy:y0 + dy + ROWS_PER_ST, dx:dx + W],
                            start=(idx == 0), stop=(idx == KT * NTAP - 1))
                        idx += 1
                at = ev_pool.tile([P, NF], FP32, tag="A")
                nc.scalar.activation(
                    at[:], xf[mt][:, st * NF:(st + 1) * NF], ACTF.Identity,
                    bias=bias2_t[mt][:], scale=inv_t[mt][:])
                rt = ev_pool.tile([P, NF], FP32, tag="r")
                nc.vector.scalar_tensor_tensor(
                    rt[:], pv[:], s1[mt][:], at[:],
                    op0=ALU.mult, op1=ALU.add)
                o_t = ev_pool.tile([P, NF], FP32, tag="o")
                nc.scalar.activation(o_t[:], rt[:], ACTF.Relu)
                nc.sync.dma_start(
                    out[b, mt * P:(mt + 1) * P].rearrange(
                        "c h w -> c (h w)")[:, st * NF:(st + 1) * NF],
                    o_t[:])


def build_nc(repeat: int = 1):
    nc = bacc.Bacc("TRN2", target_bir_lowering=False, debug=False,
                   num_devices=NCORES)
    x = nc.dram_tensor("x", [BPC, C, H, W], FP32, kind="ExternalInput").ap()
    wq = nc.dram_tensor("wq", [C, C, 3, 3], FP32, kind="ExternalInput").ap()
    wk = nc.dram_tensor("wk", [C, C, 3, 3], FP32, kind="ExternalInput").ap()
    wv = nc.dram_tensor("wv", [C, C, 3, 3], FP32, kind="ExternalInput").ap()
    gamma = nc.dram_tensor("gamma", [C], FP32, kind="ExternalInput").ap()
    beta = nc.dram_tensor("beta", [C], FP32, kind="ExternalInput").ap()
    rmean = nc.dram_tensor("running_mean", [C], FP32, kind="ExternalInput").ap()
    rvar = nc.dram_tensor("running_var", [C], FP32, kind="ExternalInput").ap()
    out = nc.dram_tensor("out", [BPC, C, H, W], FP32, kind="ExternalOutput").ap()
    with tile.TileContext(nc) as tc, ExitStack() as ctx:
        for _ in range(repeat):
            with ExitStack() as rep_ctx:
                _emit(rep_ctx, tc, nc, x, wq, wk, wv, gamma, beta, rmean,
                      rvar, out)
    nc.compile()
    return nc


_NC_CACHE = None


def _get_nc():
    global _NC_CACHE
    if _NC_CACHE is None:
        _NC_CACHE = build_nc()
    return _NC_CACHE


def make_in_maps(inputs: dict) -> list:
    rep = {k: np.ascontiguousarray(np.asarray(inputs[k], dtype=np.float32))
           for k in ("wq", "wk", "wv", "gamma", "beta",
                     "running_mean", "running_var")}
    xfull = np.ascontiguousarray(np.asarray(inputs["x"], dtype=np.float32))
    in_maps = []
    for c in range(NCORES):
        m = dict(rep)
        m["x"] = xfull[c * BPC:(c + 1) * BPC]
        in_maps.append(m)
    return in_maps


def kernel(**inputs) -> np.ndarray:
    nc = _get_nc()
    in_maps = make_in_maps(inputs)
    res = bass_utils.run_bass_kernel_spmd(nc, in_maps,
                                          core_ids=list(range(NCORES)))
    return np.concatenate([res.results[c]["out"] for c in range(NCORES)],
                          axis=0).astype(np.float32)


# revision 9
# speedup vs baseline: 2616.7630x; 41.4990x over previous
"""Trainium2 Bass kernel for nn_CSA_ConvBlock (conv-self-attention block).

Reference math (B,C,H,W = 16,256,64,64):
  fq = conv3x3(x, wq); fk = conv3x3(x, wk); fv = conv3x3(x, wv)
  q_sum = fq.sum(H); k_sum = fk.sum(C,H)
  f_scores[b,c] = sum_w q_sum[b,c,w]*k_sum[b,w] / (sqrt(W)*H^2)
  scores = softmax_C(f_scores)
  out = relu(BN_eval(scores*fv + x))

Key algebraic reduction: fq and fk are only consumed through H-sums, and
conv is linear, so q_sum/k_sum collapse to 3-tap-x-3-dy matmuls over the
column sums of x (with top/bottom row edge corrections for SAME padding).
Only conv(x, wv) is computed in full.  Since scores ~ 1/C ~ 0.004, the
attention branch is strongly suppressed relative to the fp32 residual x,
so bf16 matmuls are numerically safe.

Sharding: data-parallel over batch, 2 batches per core on 8 cores.
"""

import os
import sys
import numpy as np
from contextlib import ExitStack

if "/opt/trn_rl_repo" not in sys.path and not any(
    "trn_rl_repo" in p for p in sys.path
):
    sys.path.insert(0, "/opt/trn_rl_repo")

import concourse.bass as bass
import concourse.tile as tile
from concourse import bacc, mybir
from concourse import bass_utils
from concourse.masks import make_identity

B, C, H, W = 16, 256, 64, 64
NCORES = 8
BPC = B // NCORES          # batches per core
P = 128                    # partitions
KT = C // P                # channel k-tiles (2)
MT = C // P                # channel m-tiles (2)
PW = W + 2                 # padded width 66
PH = H + 2                 # padded height 66
NTAP = 9
ROWS_PER_ST = 8
NF = ROWS_PER_ST * W       # 512 free elems per spatial tile
ST = (H * W) // NF         # 8 spatial tiles per (batch, mtile)
EPS = 1e-5
SCORE_SCALE = 1.0 / (np.sqrt(np.float32(W)) * (H * H))  # 1/32768

FP32 = mybir.dt.float32
BF16 = mybir.dt.bfloat16
AX = mybir.AxisListType
ALU = mybir.AluOpType
ACTF = mybir.ActivationFunctionType


def _emit(ctx: ExitStack, tc: "tile.TileContext", nc, x, wq, wk, wv,
          gamma, beta, rmean, rvar, out):
    consts = ctx.enter_context(tc.tile_pool(name="consts", bufs=1))

    ident = consts.tile([P, P], FP32, tag="ident")
    make_identity(nc, ident[:])
    ones_col = consts.tile([P, 1], FP32, tag="ones")
    nc.gpsimd.memset(ones_col[:], 1.0)

    # ---- BN params to per-partition layout, fold into affine ----
    # inv = gamma / sqrt(var + eps);  bias2 = beta - mean*inv
    par = {}
    for name, ap in [("gamma", gamma), ("beta", beta),
                     ("mean", rmean), ("var", rvar)]:
        par[name] = []
        for mt in range(MT):
            t = consts.tile([P, 1], FP32, tag=f"{name}{mt}")
            nc.sync.dma_start(t[:], ap[mt * P:(mt + 1) * P])
            par[name].append(t)
    eps_t = consts.tile([P, 1], FP32, tag="eps")
    nc.gpsimd.memset(eps_t[:], EPS)
    inv_t, bias2_t = [], []
    for mt in range(MT):
        sq = consts.tile([P, 1], FP32, tag=f"sq{mt}")
        nc.scalar.activation(sq[:], par["var"][mt][:], ACTF.Sqrt, bias=eps_t[:])
        rec = consts.tile([P, 1], FP32, tag=f"rec{mt}")
        nc.vector.reciprocal(rec[:], sq[:])
        iv = consts.tile([P, 1], FP32, tag=f"inv{mt}")
        nc.vector.tensor_mul(iv[:], rec[:], par["gamma"][mt][:])
        inv_t.append(iv)
        mi = consts.tile([P, 1], FP32, tag=f"mi{mt}")
        nc.vector.tensor_mul(mi[:], par["mean"][mt][:], iv[:])
        b2 = consts.tile([P, 1], FP32, tag=f"b2{mt}")
        nc.vector.tensor_sub(b2[:], par["beta"][mt][:], mi[:])
        bias2_t.append(b2)

    # ---- weight prep ----
    # wqT/wvT: per k-tile [i=128, (tap, o=256)] bf16 (PE-transposed per tap)
    # wks:     per k-tile [i=128, tap] bf16 -- wk summed over output channels
    wT_pool = ctx.enter_context(tc.tile_pool(name="wT", bufs=1))
    wqT = [wT_pool.tile([P, NTAP * C], BF16, tag=f"wqT{kt}", name=f"wqT{kt}")
           for kt in range(KT)]
    wvT = [wT_pool.tile([P, NTAP * C], BF16, tag=f"wvT{kt}", name=f"wvT{kt}")
           for kt in range(KT)]
    wks = [wT_pool.tile([P, NTAP], BF16, tag=f"wks{kt}", name=f"wks{kt}")
           for kt in range(KT)]

    with tc.tile_pool(name="wraw", bufs=3) as wraw_pool, \
         tc.tile_pool(name="prep_psum", bufs=4, space="PSUM") as prep_psum:
        for wap, dest in [(wq, wqT), (wv, wvT)]:
            for ot in range(MT):
                raw = wraw_pool.tile([P, C * NTAP], FP32, tag="wraw")
                nc.sync.dma_start(raw[:], wap[ot * P:(ot + 1) * P])
                r3 = raw[:].rearrange("o (i t) -> o i t", t=NTAP)
                for kt in range(KT):
                    for tap in range(NTAP):
                        pt = prep_psum.tile([P, P], FP32, tag="tp")
                        nc.tensor.transpose(
                            pt[:], r3[:, kt * P:(kt + 1) * P, tap], ident[:])
                        nc.vector.tensor_copy(
                            dest[kt][:, tap * C + ot * P: tap * C + (ot + 1) * P],
                            pt[:])
        # wk: column-sum over all 256 output channels per (i, tap)
        rawk = []
        for ot in range(MT):
            rk = wraw_pool.tile([P, C * NTAP], FP32, tag="wraw")
            nc.sync.dma_start(rk[:], wk[ot * P:(ot + 1) * P])
            rawk.append(rk)
        for kt in range(KT):
            for tap in range(NTAP):
                pk = prep_psum.tile([P, 1], FP32, tag="pk")
                for ot in range(MT):
                    r3 = rawk[ot][:].rearrange("o (i t) -> o i t", t=NTAP)
                    nc.tensor.matmul(
                        pk[:], r3[:, kt * P:(kt + 1) * P, tap], ones_col[:],
                        start=(ot == 0), stop=(ot == MT - 1))
                nc.vector.tensor_copy(wks[kt][:, tap:tap + 1], pk[:])

    # ---- main per-batch pipeline ----
    xf32_pool = ctx.enter_context(tc.tile_pool(name="xf32", bufs=2 * KT))
    xpad_pool = ctx.enter_context(tc.tile_pool(name="xpad", bufs=2 * KT))
    agg_pool = ctx.enter_context(tc.tile_pool(name="agg", bufs=2 * KT))
    small = ctx.enter_context(tc.tile_pool(name="small", bufs=2))
    ev_pool = ctx.enter_context(tc.tile_pool(name="ev", bufs=3))
    qk_psum = ctx.enter_context(tc.tile_pool(name="qk_psum", bufs=2, space="PSUM"))
    misc_psum = ctx.enter_context(tc.tile_pool(name="misc_psum", bufs=1, space="PSUM"))
    fv_psum = ctx.enter_context(tc.tile_pool(name="fv_psum", bufs=4, space="PSUM"))

    for b in range(BPC):
        # load x (fp32, flat) per k-tile
        xf = []
        for kt in range(KT):
            t = xf32_pool.tile([P, H * W], FP32, tag="xf32")
            nc.sync.dma_start(t[:], x[b, kt * P:(kt + 1) * P])
            xf.append(t)

        # padded bf16 image + column-sum aggregates
        xp, aggs = [], []
        for kt in range(KT):
            tp = xpad_pool.tile([P, PH * PW], BF16, tag="xpad")
            nc.gpsimd.memset(tp[:], 0.0)
            nc.vector.tensor_copy(
                tp[:].rearrange("p (r c) -> p r c", c=PW)[:, 1:H + 1, 1:W + 1],
                xf[kt][:].rearrange("p (h w) -> p h w", w=W))
            xp.append(tp)

            cs = small.tile([P, W], FP32, tag="cs")
            nc.vector.tensor_reduce(
                cs[:], xf[kt][:].rearrange("p (h w) -> p w h", w=W),
                axis=AX.X, op=ALU.add)
            xhw = xf[kt][:].rearrange("p (h w) -> p h w", w=W)
            ag = agg_pool.tile([P, 3 * PW], BF16, tag="agg")
            nc.gpsimd.memset(ag[:], 0.0)
            a3 = ag[:].rearrange("p (a c) -> p a c", c=PW)
            # dy=0 row-window is rows -1..H-2: colsum - bottom row
            nc.vector.tensor_sub(a3[:, 0, 1:W + 1], cs[:], xhw[:, H - 1, :])
            nc.vector.tensor_copy(a3[:, 1, 1:W + 1], cs[:])
            # dy=2 row-window is rows 1..H: colsum - top row
            nc.vector.tensor_sub(a3[:, 2, 1:W + 1], cs[:], xhw[:, 0, :])
            aggs.append(ag)

        # q_sum (per mtile) and k_sum into one PSUM tile:
        # cols [mt*W:(mt+1)*W] = q_sum[mt]; row 0 cols [2W:3W] = k_sum
        qk = qk_psum.tile([P, 3 * W], FP32, tag="qk")
        for mt in range(MT):
            idx = 0
            for kt in range(KT):
                a3 = aggs[kt][:].rearrange("p (a c) -> p a c", c=PW)
                for tap in range(NTAP):
                    dy, dx = divmod(tap, 3)
                    nc.tensor.matmul(
                        qk[:, mt * W:(mt + 1) * W],
                        wqT[kt][:, tap * C + mt * P: tap * C + mt * P + P],
                        a3[:, dy, dx:dx + W],
                        start=(idx == 0), stop=(idx == KT * NTAP - 1))
                    idx += 1
        idx = 0
        for kt in range(KT):
            a3 = aggs[kt][:].rearrange("p (a c) -> p a c", c=PW)
            for tap in range(NTAP):
                dy, dx = divmod(tap, 3)
                nc.tensor.matmul(
                    qk[0:1, 2 * W:3 * W],
                    wks[kt][:, tap:tap + 1],
                    a3[:, dy, dx:dx + W],
                    start=(idx == 0), stop=(idx == KT * NTAP - 1))
                idx += 1

        # f_scores[c] = dot(q_sum[c,:], k_sum) (scale folded into softmax)
        ksb = small.tile([1, W], FP32, tag="ksb")
        nc.vector.tensor_copy(ksb[:], qk[0:1, 2 * W:3 * W])
        kb = small.tile([P, W], FP32, tag="kb")
        nc.gpsimd.partition_broadcast(kb[:], ksb[:])
        fs = []
        for mt in range(MT):
            scr = small.tile([P, W], FP32, tag=f"scr{mt}")
            nc.vector.tensor_mul(scr[:], qk[:, mt * W:(mt + 1) * W], kb[:])
            f = small.tile([P, 1], FP32, tag=f"fs{mt}")
            nc.vector.tensor_reduce(f[:], scr[:], axis=AX.X, op=ALU.add)
            fs.append(f)

        # softmax over C=256 in a [1, 256] row
        fsrow = misc_psum.tile([1, C], FP32, tag="fsrow")
        for mt in range(MT):
            nc.tensor.transpose(fsrow[0:1, mt * P:(mt + 1) * P], fs[mt][:], ident[:])
        mx = small.tile([1, 1], FP32, tag="mx")
        nc.vector.tensor_reduce(mx[:], fsrow[:], axis=AX.X, op=ALU.max)
        mxs = small.tile([1, 1], FP32, tag="mxs")
        nc.vector.tensor_scalar_mul(mxs[:], mx[:], -float(SCORE_SCALE))
        es = small.tile([1, C], FP32, tag="es")
        nc.scalar.activation(es[:], fsrow[:], ACTF.Exp,
                             bias=mxs[:], scale=float(SCORE_SCALE))
        ssum = small.tile([1, 1], FP32, tag="ssum")
        nc.vector.tensor_reduce(ssum[:], es[:], axis=AX.X, op=ALU.add)
        rs = small.tile([1, 1], FP32, tag="rs")
        nc.vector.reciprocal(rs[:], ssum[:])
        srow = small.tile([1, C], FP32, tag="srow")
        nc.vector.tensor_scalar_mul(srow[:], es[:], rs[:])

        # scores back to [128,1] per mtile (K=1 matmul), fold in BN inv
        s1 = []
        for mt in range(MT):
            stp = misc_psum.tile([P, 1], FP32, tag="stp")
            nc.tensor.matmul(stp[:], srow[:, mt * P:(mt + 1) * P],
                             ones_col[0:1, 0:1], start=True, stop=True)
            t = small.tile([P, 1], FP32, tag=f"s1{mt}")
            nc.vector.tensor_mul(t[:], stp[:], inv_t[mt][:])
            s1.append(t)

        # fv conv (18 accumulating matmuls per [128,512] tile) + fused
        # eviction: out = relu(fv*s1 + (x*inv + bias2))
        for mt in range(MT):
            for st in range(ST):
                y0 = st * ROWS_PER_ST
                pv = fv_psum.tile([P, NF], FP32, tag="fv")
                idx = 0
                for kt in range(KT):
                    x3 = xp[kt][:].rearrange("p (r c) -> p r c", c=PW)
                    for tap in range(NTAP):
                        dy, dx = divmod(tap, 3)
                        nc.tensor.matmul(
                            pv[:],
                            wvT[kt][:, tap * C + mt * P: tap * C + mt * P + P],
                            x3[:, y0 + dy:y0 + dy + ROWS_PER_ST, dx:dx + W],
                            start=(idx == 0), stop=(idx == KT * NTAP - 1))
                        idx += 1
                at = ev_pool.tile([P, NF], FP32, tag="A")
                nc.scalar.activation(
                    at[:], xf[mt][:, st * NF:(st + 1) * NF], ACTF.Identity,
                    bias=bias2_t[mt][:], scale=inv_t[mt][:])
                rt = ev_pool.tile([P, NF], FP32, tag="r")
                nc.vector.scalar_tensor_tensor(
                    rt[:], pv[:], s1[mt][:], at[:],
                    op0=ALU.mult, op1=ALU.add)
                o_t = ev_pool.tile([P, NF], FP32, tag="o")
                nc.scalar.activation(o_t[:], rt[:], ACTF.Relu)
                nc.sync.dma_start(
                    out[b, mt * P:(mt + 1) * P].rearrange(
                        "c h w -> c (h w)")[:, st * NF:(st + 1) * NF],
                    o_t[:])


def build_nc(repeat: int = 1, loop_n: int | None = None):
    nc = bacc.Bacc("TRN2", target_bir_lowering=False, debug=False,
                   num_devices=NCORES)
    x = nc.dram_tensor("x", [BPC, C, H, W], FP32, kind="ExternalInput").ap()
    wq = nc.dram_tensor("wq", [C, C, 3, 3], FP32, kind="ExternalInput").ap()
    wk = nc.dram_tensor("wk", [C, C, 3, 3], FP32, kind="ExternalInput").ap()
    wv = nc.dram_tensor("wv", [C, C, 3, 3], FP32, kind="ExternalInput").ap()
    gamma = nc.dram_tensor("gamma", [C], FP32, kind="ExternalInput").ap()
    beta = nc.dram_tensor("beta", [C], FP32, kind="ExternalInput").ap()
    rmean = nc.dram_tensor("running_mean", [C], FP32, kind="ExternalInput").ap()
    rvar = nc.dram_tensor("running_var", [C], FP32, kind="ExternalInput").ap()
    out = nc.dram_tensor("out", [BPC, C, H, W], FP32, kind="ExternalOutput").ap()
    with tile.TileContext(nc) as tc, ExitStack() as ctx:
        if loop_n is not None:
            with tc.For_i(0, loop_n, 1):
                with ExitStack() as rep_ctx:
                    _emit(rep_ctx, tc, nc, x, wq, wk, wv, gamma, beta,
                          rmean, rvar, out)
        else:
            for _ in range(repeat):
                with ExitStack() as rep_ctx:
                    _emit(rep_ctx, tc, nc, x, wq, wk, wv, gamma, beta,
                          rmean, rvar, out)
    nc.compile()
    return nc


_NC_CACHE = None


def _get_nc():
    global _NC_CACHE
    if _NC_CACHE is None:
        _NC_CACHE = build_nc()
    return _NC_CACHE


def make_in_maps(inputs: dict) -> list:
    rep = {k: np.ascontiguousarray(np.asarray(inputs[k], dtype=np.float32))
           for k in ("wq", "wk", "wv", "gamma", "beta",
                     "running_mean", "running_var")}
    xfull = np.ascontiguousarray(np.asarray(inputs["x"], dtype=np.float32))
    in_maps = []
    for c in range(NCORES):
        m = dict(rep)
        m["x"] = xfull[c * BPC:(c + 1) * BPC]
        in_maps.append(m)
    return in_maps


def kernel(**inputs) -> np.ndarray:
    nc = _get_nc()
    in_maps = make_in_maps(inputs)
    res = bass_utils.run_bass_kernel_spmd(nc, in_maps,
                                          core_ids=list(range(NCORES)))
    return np.concatenate([res.results[c]["out"] for c in range(NCORES)],
                          axis=0).astype(np.float32)
